# revision 36
# baseline (speedup 1.0000x reference)
"""MoE layer (E=8 experts, top-2 routing) on 8 Trainium2 NeuronCores.

Strategy: expert-parallel. The host computes the (tiny) gating network in
fp64 -- logits = x @ wg + bg, top-2, softmax -- and dispatches each token to
the cores owning its two selected experts (the "all-to-all dispatch tokens
by routing decision" sharding). Each core runs its expert's FFN
    y = relu(x_e @ w1[e] + b1[e]) @ w2[e]
over its gathered tokens (padded to a common static NT), scales rows by the
gate weight on-device, and the host scatter-adds the two slots per token
back together (plus the combine@b2 bias term).

All device inputs are host-permuted so that every SBUF partition's data is
one contiguous DRAM run (~8KB DMA lines -> 4KB packets -> high per-engine
DMA rate); without this the HWDGE engines move ~13 GB/s each on 1KB lines.

Hardcoded problem shape: x [4,4096,512], w1 [8,512,1024], w2 [8,1024,512],
wg [512,8], top_k=2.
"""

import os
import numpy as np

B, S, D, F, E = 4, 4096, 512, 1024, 8
TOP_K = 2
N_CORES = 8
KD = D // 128   # contraction blocks for mm1
FB = F // 128   # F blocks (h partition blocks / mm2 contraction blocks)

# matmul operand dtype mode: "bf16" | "f32" | "f32r"
DT_MODE = os.environ.get("MOE_DT", "bf16")
TRACE = os.environ.get("MOE_TRACE", "0") == "1"


def _chunk_plan(NT, mode):
    """Token chunk sizes (moving-dim tiles) and DMA groups.

    f32r matmuls with moving dim < 256 drop to 4 cycles/row, so keep
    chunks >= 256 there; bf16 has no such penalty and a small LAST chunk
    trims the post-last-matmul tail. Group 0 is a single chunk so the
    first x load is small (PE starts sooner); later groups pair chunks
    for fat DMA transfers.
    """
    rem = NT % 512
    chunks = [512] * (NT // 512)
    if mode == "bf16":
        if rem:
            chunks.append(rem)
    else:
        if rem == 128:
            chunks = chunks[:-1] + [384, 256]
        elif rem:
            chunks.append(rem)
        assert all(c >= 256 for c in chunks), chunks
    assert sum(chunks) == NT, chunks
    groups = [[0]]  # list of lists of chunk indices
    for i in range(1, len(chunks), 2):
        groups.append(list(range(i, min(i + 2, len(chunks)))))
    return chunks, groups


def _build_program(NT, mode):
    from concourse import bacc, tile, mybir

    dt = mybir.dt
    DT = {"bf16": dt.bfloat16, "f32": dt.float32, "f32r": dt.float32r}[mode]

    nc = bacc.Bacc("TRN2", target_bir_lowering=False, debug=False)

    chunks, groups = _chunk_plan(NT, mode)
    offs = [sum(chunks[:i]) for i in range(len(chunks) + 1)]

    # host-permuted inputs: per-partition contiguous runs
    xp_d = nc.dram_tensor("xp", [128, KD * NT], DT, kind="ExternalInput").ap()
    w1_d = nc.dram_tensor("w1p", [128, KD * F], DT, kind="ExternalInput").ap()
    w2_d = nc.dram_tensor("w2p", [128, FB * D], DT, kind="ExternalInput").ap()
    b1_d = nc.dram_tensor("b1c", [128, FB], dt.float32, kind="ExternalInput").ap()
    g_d = nc.dram_tensor("gate2", [128, NT // 128], dt.float32, kind="ExternalInput").ap()
    y_d = nc.dram_tensor("y", [NT, D], dt.float32, kind="ExternalOutput").ap()

    def psplit_dma(out_ap, in_ap, n, eng=None):
        # split a [128, W] transfer into n partition slices so several DMA
        # engines work it concurrently; eng picks the issuing engine's
        # HWDGE queue (sync and scalar are separate queue domains on TRN2)
        eng = eng or nc.sync
        step = 128 // n
        for j in range(n):
            eng.dma_start(
                out=out_ap[j * step:(j + 1) * step, :],
                in_=in_ap[j * step:(j + 1) * step, :],
            )

    with tile.TileContext(nc) as tc:
        with (
            tc.tile_pool(name="w", bufs=1) as wpool,
            tc.tile_pool(name="x", bufs=2) as xpool,
            tc.tile_pool(name="h", bufs=3) as hpool,
            tc.tile_pool(name="o", bufs=4) as opool,
            tc.tile_pool(name="ps1", bufs=4, space="PSUM") as ps1,
            tc.tile_pool(name="ps2", bufs=4, space="PSUM") as ps2,
        ):
            # DMA issue order = FIFO service order: w1 + x group 0 (needed
            # by the first matmuls), then w2, then later x groups. Each
            # split transfer is served by one ~14 GB/s engine and earlier-
            # queued transfers grab engines first, so startup-critical loads
            # are split 8-way (w1+x0 -> all 16 engines -> ~224 GB/s), while
            # steady-state x prefetch stays 4-way (plenty of headroom).
            w1_sb = wpool.tile([128, KD * F], DT)
            psplit_dma(w1_sb, w1_d, 8)

            x_tiles = {}

            def load_group(g):
                cidx = groups[g]
                goff = offs[cidx[0]]
                gs = sum(chunks[c] for c in cidx)
                x_sb = xpool.tile([128, KD * gs], DT, tag="x")
                psplit_dma(x_sb, xp_d[:, KD * goff:KD * (goff + gs)], 8 if g <= 1 else 4)
                x_tiles[g] = x_sb

            load_group(0)

            w2_sb = wpool.tile([128, FB * D], DT)
            psplit_dma(w2_sb, w2_d, 8)
            b1_sb = wpool.tile([128, FB], dt.float32)
            nc.sync.dma_start(out=b1_sb[:], in_=b1_d[:])
            g_sb = wpool.tile([128, NT // 128], dt.float32)
            nc.sync.dma_start(out=g_sb[:], in_=g_d[:])

            for g, cidx in enumerate(groups):
                if g + 1 < len(groups):
                    load_group(g + 1)
                x_sb = x_tiles.pop(g)
                goff = offs[cidx[0]]
                gs = sum(chunks[c] for c in cidx)
                for c in cidx:
                    cs = chunks[c]
                    off = offs[c]
                    lo = off - goff  # chunk's token offset inside the group
                    h_sb = hpool.tile([128, FB, cs], DT, tag="h")
                    for fb in range(FB):
                        p = ps1.tile([128, cs], dt.float32, tag="ps1")
                        for kc in range(KD):
                            nc.tensor.matmul(
                                p[:],
                                w1_sb[:, kc * F + fb * 128:kc * F + (fb + 1) * 128],
                                x_sb[:, kc * gs + lo:kc * gs + lo + cs],
                                start=(kc == 0),
                                stop=(kc == KD - 1),
                            )
                        nc.scalar.activation(
                            h_sb[:, fb, :],
                            p[:],
                            mybir.ActivationFunctionType.Relu,
                            bias=b1_sb[:, fb:fb + 1],
                            scale=1.0,
                        )
                    for tb in range(cs // 128):
                        p2 = ps2.tile([128, 512], dt.float32, tag="ps2")
                        for fb in range(FB):
                            nc.tensor.matmul(
                                p2[:],
                                h_sb[:, fb, tb * 128:(tb + 1) * 128],
                                w2_sb[:, fb * D:(fb + 1) * D],
                                start=(fb == 0),
                                stop=(fb == FB - 1),
                            )
                        o_sb = opool.tile([128, 512], dt.float32, tag="o")
                        nc.vector.tensor_scalar_mul(
                            o_sb[:], p2[:], g_sb[:, off // 128 + tb:off // 128 + tb + 1]
                        )
                        nc.sync.dma_start(
                            out=y_d[off + tb * 128:off + (tb + 1) * 128, :],
                            in_=o_sb[:],
                        )
    nc.compile()
    return nc


def _install_ntff_hook():
    """Register the axon NTFF profiling hook that run_bass_kernel_spmd
    (trace=True) looks for under antenv.axon_hooks; this container's antenv
    lacks that module, so recreate it via ctypes against libaxon_pjrt.so."""
    import sys, types, ctypes, contextlib

    if "antenv.axon_hooks" in sys.modules:
        return
    try:
        lib = ctypes.CDLL("/opt/axon/libaxon_pjrt.so")
    except OSError:
        return
    if not hasattr(lib, "axon_start_nrt_profile"):
        return
    lib.axon_start_nrt_profile.argtypes = [ctypes.POINTER(ctypes.c_int64), ctypes.c_size_t]
    lib.axon_start_nrt_profile.restype = ctypes.c_int64
    lib.axon_stop_nrt_profile.argtypes = [ctypes.c_char_p]
    lib.axon_stop_nrt_profile.restype = ctypes.c_int64

    @contextlib.contextmanager
    def _hook(output_dir, device_ids):
        import jax

        jax.devices()
        if device_ids:
            ids = (ctypes.c_int64 * len(device_ids))(*device_ids)
            rc = lib.axon_start_nrt_profile(ids, len(device_ids))
        else:
            rc = lib.axon_start_nrt_profile(None, 0)
        if rc != 0:
            raise RuntimeError(f"axon_start_nrt_profile rc={rc}")
        try:
            yield
        finally:
            n = lib.axon_stop_nrt_profile(str(output_dir).encode())
            print(f"profile: {n} ntff file(s) written to {output_dir}")

    mod = types.ModuleType("antenv.axon_hooks")
    _holder = {"h": _hook}
    mod.set_axon_ntff_profile_hook = lambda h: _holder.__setitem__("h", h)
    mod.get_axon_ntff_profile_hook = lambda: _holder["h"]
    sys.modules["antenv.axon_hooks"] = mod

    # avoid the S3/Fish artifact upload in the trace post-processing path
    import concourse.bass_utils as bu

    bu.upload_artifacts = lambda tmpdir: str(tmpdir)


def kernel(**inputs):
    from concourse.bass_utils import run_bass_kernel_spmd

    if TRACE:
        _install_ntff_hook()

    x = np.asarray(inputs["x"], np.float32)
    w1 = np.asarray(inputs["w1"], np.float32)
    b1 = np.asarray(inputs["b1"], np.float32)
    w2 = np.asarray(inputs["w2"], np.float32)
    b2 = np.asarray(inputs["b2"], np.float32)
    wg = np.asarray(inputs["wg"], np.float32)
    bg = np.asarray(inputs["bg"], np.float32)

    T = x.shape[0] * x.shape[1]
    xf = x.reshape(T, D)

    # ---- host gating (fp64): logits -> top-2 (jax.lax.top_k tie order:
    # lower index wins -> stable argsort on -logits) -> softmax over top-2.
    logits = xf.astype(np.float64) @ wg.astype(np.float64) + bg.astype(np.float64)
    order = np.argsort(-logits, axis=1, kind="stable")
    top_idx = order[:, :TOP_K]                      # [T, K]
    top_vals = np.take_along_axis(logits, top_idx, axis=1)
    gwts = np.exp(top_vals - top_vals.max(axis=1, keepdims=True))
    gwts = gwts / gwts.sum(axis=1, keepdims=True)   # [T, K]

    # ---- dispatch: sort slots (t, k) by expert; per-expert contiguous runs.
    flat_expert = top_idx.ravel()                   # slot s = t*K + k
    perm = np.argsort(flat_expert, kind="stable")   # slots grouped by expert
    counts = np.bincount(flat_expert, minlength=E)
    cum = np.concatenate([[0], np.cumsum(counts)])
    slot_tok = perm // TOP_K                        # token of each sorted slot
    gates_sorted = gwts.ravel()[perm].astype(np.float32)

    NT = max(512, int(-(-counts.max() // 128)) * 128)
    NTG = NT // 128
    chunks, groups = _chunk_plan(NT, DT_MODE)
    offs = [sum(chunks[:i]) for i in range(len(chunks) + 1)]

    if DT_MODE == "bf16":
        import ml_dtypes
        io_dtype = ml_dtypes.bfloat16
    else:
        io_dtype = np.float32

    w1_io = w1.astype(io_dtype)
    w2_io = w2.astype(io_dtype)

    group_bounds = [(offs[ci[0]], offs[ci[-1] + 1]) for ci in groups]

    def permute_x(xt):
        # xt [D, NT] -> [128, sum_g KD*gs]: per partition, per group,
        # (kc, token) contiguous
        xr = xt.reshape(KD, 128, NT)
        parts = [
            xr[:, :, g0:g1].transpose(1, 0, 2).reshape(128, -1)
            for (g0, g1) in group_bounds
        ]
        return np.ascontiguousarray(np.concatenate(parts, axis=1))

    in_maps = []
    for e in range(E):
        n = int(counts[e])
        toks = slot_tok[cum[e]:cum[e] + n]
        xt = np.zeros((D, NT), io_dtype)
        xt[:, :n] = xf[toks].astype(io_dtype).T
        gate = np.zeros(NT, np.float32)
        gate[:n] = gates_sorted[cum[e]:cum[e] + n]
        in_maps.append({
            "xp": permute_x(xt),
            "w1p": np.ascontiguousarray(
                w1_io[e].reshape(KD, 128, F).transpose(1, 0, 2).reshape(128, KD * F)),
            "w2p": np.ascontiguousarray(
                w2_io[e].reshape(FB, 128, D).transpose(1, 0, 2).reshape(128, FB * D)),
            "b1c": np.ascontiguousarray(b1[e].reshape(FB, 128).T),
            "gate2": np.ascontiguousarray(gate.reshape(NTG, 128).T),
        })

    nc = _build_program(NT, DT_MODE)
    res = run_bass_kernel_spmd(nc, in_maps, list(range(N_CORES)), trace=TRACE)
    if TRACE and res.exec_time_ns is not None:
        print(f"HW exec time: {res.exec_time_ns} ns")

    # ---- unshard: scatter slots back, sum the K slots per token, add b2 term.
    out_slots = np.zeros((T * TOP_K, D), np.float32)
    for e in range(E):
        n = int(counts[e])
        out_slots[perm[cum[e]:cum[e] + n]] = res.results[e]["y"][:n]
    out = out_slots.reshape(T, TOP_K, D).sum(axis=1)

    # combine @ b2 (gate-weighted expert output biases)
    combine = np.zeros((T, E), np.float32)
    np.put_along_axis(combine, top_idx, gwts.astype(np.float32), axis=1)
    out += combine @ b2

    return out.reshape(B, S, D).astype(np.float32)


# revision 37
# speedup vs baseline: 1.0797x; 1.0797x over previous
"""MoE layer (E=8 experts, top-2 routing) on 8 Trainium2 NeuronCores.

Strategy: expert-parallel. The host computes the (tiny) gating network in
fp64 -- logits = x @ wg + bg, top-2, softmax -- and dispatches each token to
the cores owning its two selected experts (the "all-to-all dispatch tokens
by routing decision" sharding). Each core runs its expert's FFN
    y = relu(x_e @ w1[e] + b1[e]) @ w2[e]
over its gathered tokens (padded to a common static NT), scales rows by the
gate weight on-device, and the host scatter-adds the two slots per token
back together (plus the combine@b2 bias term).

All device inputs are host-permuted so that every SBUF partition's data is
one contiguous DRAM run (~8KB DMA lines -> 4KB packets -> high per-engine
DMA rate); without this the HWDGE engines move ~13 GB/s each on 1KB lines.

Hardcoded problem shape: x [4,4096,512], w1 [8,512,1024], w2 [8,1024,512],
wg [512,8], top_k=2.
"""

import os
import numpy as np

B, S, D, F, E = 4, 4096, 512, 1024, 8
TOP_K = 2
N_CORES = 8
KD = D // 128   # contraction blocks for mm1
FB = F // 128   # F blocks (h partition blocks / mm2 contraction blocks)

# matmul operand dtype mode: "bf16" | "f32" | "f32r"
DT_MODE = os.environ.get("MOE_DT", "bf16")
TRACE = os.environ.get("MOE_TRACE", "0") == "1"


def _chunk_plan(NT, mode):
    """Token chunk sizes (moving-dim tiles) and DMA groups.

    f32r matmuls with moving dim < 256 drop to 4 cycles/row, so keep
    chunks >= 256 there; bf16 has no such penalty and a small LAST chunk
    trims the post-last-matmul tail. Group 0 is a single chunk so the
    first x load is small (PE starts sooner); later groups pair chunks
    for fat DMA transfers.
    """
    rem = NT % 512
    chunks = [512] * (NT // 512)
    if mode == "bf16":
        if rem:
            chunks.append(rem)
    else:
        if rem == 128:
            chunks = chunks[:-1] + [384, 256]
        elif rem:
            chunks.append(rem)
        assert all(c >= 256 for c in chunks), chunks
    assert sum(chunks) == NT, chunks
    groups = [[0]]  # list of lists of chunk indices
    for i in range(1, len(chunks), 2):
        groups.append(list(range(i, min(i + 2, len(chunks)))))
    return chunks, groups


def _build_program(NT, mode):
    from concourse import bacc, tile, mybir

    dt = mybir.dt
    DT = {"bf16": dt.bfloat16, "f32": dt.float32, "f32r": dt.float32r}[mode]

    nc = bacc.Bacc("TRN2", target_bir_lowering=False, debug=False)

    chunks, groups = _chunk_plan(NT, mode)
    offs = [sum(chunks[:i]) for i in range(len(chunks) + 1)]

    # host-permuted inputs: per-partition contiguous runs
    xp_d = nc.dram_tensor("xp", [128, KD * NT], DT, kind="ExternalInput").ap()
    w1_d = nc.dram_tensor("w1p", [128, KD * F], DT, kind="ExternalInput").ap()
    w2_d = nc.dram_tensor("w2p", [128, FB * D], DT, kind="ExternalInput").ap()
    b1_d = nc.dram_tensor("b1c", [128, FB], dt.float32, kind="ExternalInput").ap()
    g_d = nc.dram_tensor("gate2", [128, NT // 128], dt.float32, kind="ExternalInput").ap()
    y_d = nc.dram_tensor("y", [NT, D], dt.float32, kind="ExternalOutput").ap()

    def psplit_dma(out_ap, in_ap, n, eng=None):
        # split a [128, W] transfer into n partition slices so several DMA
        # engines work it concurrently; eng picks the issuing engine's
        # HWDGE queue (sync and scalar are separate queue domains on TRN2)
        eng = eng or nc.sync
        step = 128 // n
        for j in range(n):
            eng.dma_start(
                out=out_ap[j * step:(j + 1) * step, :],
                in_=in_ap[j * step:(j + 1) * step, :],
            )

    with tile.TileContext(nc) as tc:
        with (
            tc.tile_pool(name="w", bufs=1) as wpool,
            tc.tile_pool(name="x", bufs=2) as xpool,
            tc.tile_pool(name="h", bufs=3) as hpool,
            tc.tile_pool(name="o", bufs=4) as opool,
            tc.tile_pool(name="ps1", bufs=4, space="PSUM") as ps1,
            tc.tile_pool(name="ps2", bufs=4, space="PSUM") as ps2,
        ):
            # DMA issue order = FIFO service order: w1 + x group 0 (needed
            # by the first matmuls), then w2, then later x groups. 4-way
            # [32-partition] splits are measured optimal: ~14 GB/s per
            # engine; narrower slices drop per-engine rate (SBUF ports) and
            # wider ones leave engines idle.
            w1_sb = wpool.tile([128, KD * F], DT)
            psplit_dma(w1_sb, w1_d, 4)

            x_tiles = {}

            def load_group(g):
                cidx = groups[g]
                goff = offs[cidx[0]]
                gs = sum(chunks[c] for c in cidx)
                x_sb = xpool.tile([128, KD * gs], DT, tag="x")
                psplit_dma(x_sb, xp_d[:, KD * goff:KD * (goff + gs)], 4)
                x_tiles[g] = x_sb

            load_group(0)

            w2_sb = wpool.tile([128, FB * D], DT)
            psplit_dma(w2_sb, w2_d, 4)
            b1_sb = wpool.tile([128, FB], dt.float32)
            nc.sync.dma_start(out=b1_sb[:], in_=b1_d[:])
            g_sb = wpool.tile([128, NT // 128], dt.float32)
            nc.sync.dma_start(out=g_sb[:], in_=g_d[:])

            for g, cidx in enumerate(groups):
                if g + 1 < len(groups):
                    load_group(g + 1)
                x_sb = x_tiles.pop(g)
                goff = offs[cidx[0]]
                gs = sum(chunks[c] for c in cidx)
                for c in cidx:
                    cs = chunks[c]
                    off = offs[c]
                    lo = off - goff  # chunk's token offset inside the group
                    h_sb = hpool.tile([128, FB, cs], DT, tag="h")
                    for fb in range(FB):
                        p = ps1.tile([128, cs], dt.float32, tag="ps1")
                        for kc in range(KD):
                            nc.tensor.matmul(
                                p[:],
                                w1_sb[:, kc * F + fb * 128:kc * F + (fb + 1) * 128],
                                x_sb[:, kc * gs + lo:kc * gs + lo + cs],
                                start=(kc == 0),
                                stop=(kc == KD - 1),
                            )
                        nc.scalar.activation(
                            h_sb[:, fb, :],
                            p[:],
                            mybir.ActivationFunctionType.Relu,
                            bias=b1_sb[:, fb:fb + 1],
                            scale=1.0,
                        )
                    for tb in range(cs // 128):
                        p2 = ps2.tile([128, 512], dt.float32, tag="ps2")
                        for fb in range(FB):
                            nc.tensor.matmul(
                                p2[:],
                                h_sb[:, fb, tb * 128:(tb + 1) * 128],
                                w2_sb[:, fb * D:(fb + 1) * D],
                                start=(fb == 0),
                                stop=(fb == FB - 1),
                            )
                        o_sb = opool.tile([128, 512], dt.float32, tag="o")
                        nc.vector.tensor_scalar_mul(
                            o_sb[:], p2[:], g_sb[:, off // 128 + tb:off // 128 + tb + 1]
                        )
                        nc.sync.dma_start(
                            out=y_d[off + tb * 128:off + (tb + 1) * 128, :],
                            in_=o_sb[:],
                        )
    nc.compile()
    return nc


def _install_ntff_hook():
    """Register the axon NTFF profiling hook that run_bass_kernel_spmd
    (trace=True) looks for under antenv.axon_hooks; this container's antenv
    lacks that module, so recreate it via ctypes against libaxon_pjrt.so."""
    import sys, types, ctypes, contextlib

    if "antenv.axon_hooks" in sys.modules:
        return
    try:
        lib = ctypes.CDLL("/opt/axon/libaxon_pjrt.so")
    except OSError:
        return
    if not hasattr(lib, "axon_start_nrt_profile"):
        return
    lib.axon_start_nrt_profile.argtypes = [ctypes.POINTER(ctypes.c_int64), ctypes.c_size_t]
    lib.axon_start_nrt_profile.restype = ctypes.c_int64
    lib.axon_stop_nrt_profile.argtypes = [ctypes.c_char_p]
    lib.axon_stop_nrt_profile.restype = ctypes.c_int64

    @contextlib.contextmanager
    def _hook(output_dir, device_ids):
        import jax

        jax.devices()
        if device_ids:
            ids = (ctypes.c_int64 * len(device_ids))(*device_ids)
            rc = lib.axon_start_nrt_profile(ids, len(device_ids))
        else:
            rc = lib.axon_start_nrt_profile(None, 0)
        if rc != 0:
            raise RuntimeError(f"axon_start_nrt_profile rc={rc}")
        try:
            yield
        finally:
            n = lib.axon_stop_nrt_profile(str(output_dir).encode())
            print(f"profile: {n} ntff file(s) written to {output_dir}")

    mod = types.ModuleType("antenv.axon_hooks")
    _holder = {"h": _hook}
    mod.set_axon_ntff_profile_hook = lambda h: _holder.__setitem__("h", h)
    mod.get_axon_ntff_profile_hook = lambda: _holder["h"]
    sys.modules["antenv.axon_hooks"] = mod

    # avoid the S3/Fish artifact upload in the trace post-processing path
    import concourse.bass_utils as bu

    bu.upload_artifacts = lambda tmpdir: str(tmpdir)


def kernel(**inputs):
    from concourse.bass_utils import run_bass_kernel_spmd

    if TRACE:
        _install_ntff_hook()

    x = np.asarray(inputs["x"], np.float32)
    w1 = np.asarray(inputs["w1"], np.float32)
    b1 = np.asarray(inputs["b1"], np.float32)
    w2 = np.asarray(inputs["w2"], np.float32)
    b2 = np.asarray(inputs["b2"], np.float32)
    wg = np.asarray(inputs["wg"], np.float32)
    bg = np.asarray(inputs["bg"], np.float32)

    T = x.shape[0] * x.shape[1]
    xf = x.reshape(T, D)

    # ---- host gating (fp64): logits -> top-2 (jax.lax.top_k tie order:
    # lower index wins -> stable argsort on -logits) -> softmax over top-2.
    logits = xf.astype(np.float64) @ wg.astype(np.float64) + bg.astype(np.float64)
    order = np.argsort(-logits, axis=1, kind="stable")
    top_idx = order[:, :TOP_K]                      # [T, K]
    top_vals = np.take_along_axis(logits, top_idx, axis=1)
    gwts = np.exp(top_vals - top_vals.max(axis=1, keepdims=True))
    gwts = gwts / gwts.sum(axis=1, keepdims=True)   # [T, K]

    # ---- dispatch: sort slots (t, k) by expert; per-expert contiguous runs.
    flat_expert = top_idx.ravel()                   # slot s = t*K + k
    perm = np.argsort(flat_expert, kind="stable")   # slots grouped by expert
    counts = np.bincount(flat_expert, minlength=E)
    cum = np.concatenate([[0], np.cumsum(counts)])
    slot_tok = perm // TOP_K                        # token of each sorted slot
    gates_sorted = gwts.ravel()[perm].astype(np.float32)

    NT = max(512, int(-(-counts.max() // 128)) * 128)
    NTG = NT // 128
    chunks, groups = _chunk_plan(NT, DT_MODE)
    offs = [sum(chunks[:i]) for i in range(len(chunks) + 1)]

    if DT_MODE == "bf16":
        import ml_dtypes
        io_dtype = ml_dtypes.bfloat16
    else:
        io_dtype = np.float32

    w1_io = w1.astype(io_dtype)
    w2_io = w2.astype(io_dtype)

    group_bounds = [(offs[ci[0]], offs[ci[-1] + 1]) for ci in groups]

    def permute_x(xt):
        # xt [D, NT] -> [128, sum_g KD*gs]: per partition, per group,
        # (kc, token) contiguous
        xr = xt.reshape(KD, 128, NT)
        parts = [
            xr[:, :, g0:g1].transpose(1, 0, 2).reshape(128, -1)
            for (g0, g1) in group_bounds
        ]
        return np.ascontiguousarray(np.concatenate(parts, axis=1))

    in_maps = []
    for e in range(E):
        n = int(counts[e])
        toks = slot_tok[cum[e]:cum[e] + n]
        xt = np.zeros((D, NT), io_dtype)
        xt[:, :n] = xf[toks].astype(io_dtype).T
        gate = np.zeros(NT, np.float32)
        gate[:n] = gates_sorted[cum[e]:cum[e] + n]
        in_maps.append({
            "xp": permute_x(xt),
            "w1p": np.ascontiguousarray(
                w1_io[e].reshape(KD, 128, F).transpose(1, 0, 2).reshape(128, KD * F)),
            "w2p": np.ascontiguousarray(
                w2_io[e].reshape(FB, 128, D).transpose(1, 0, 2).reshape(128, FB * D)),
            "b1c": np.ascontiguousarray(b1[e].reshape(FB, 128).T),
            "gate2": np.ascontiguousarray(gate.reshape(NTG, 128).T),
        })

    nc = _build_program(NT, DT_MODE)
    res = run_bass_kernel_spmd(nc, in_maps, list(range(N_CORES)), trace=TRACE)
    if TRACE and res.exec_time_ns is not None:
        print(f"HW exec time: {res.exec_time_ns} ns")

    # ---- unshard: scatter slots back, sum the K slots per token, add b2 term.
    out_slots = np.zeros((T * TOP_K, D), np.float32)
    for e in range(E):
        n = int(counts[e])
        out_slots[perm[cum[e]:cum[e] + n]] = res.results[e]["y"][:n]
    out = out_slots.reshape(T, TOP_K, D).sum(axis=1)

    # combine @ b2 (gate-weighted expert output biases)
    combine = np.zeros((T, E), np.float32)
    np.put_along_axis(combine, top_idx, gwts.astype(np.float32), axis=1)
    out += combine @ b2

    return out.reshape(B, S, D).astype(np.float32)


# revision 39
# speedup vs baseline: 1.0827x; 1.0028x over previous
"""MoE layer (E=8 experts, top-2 routing) on 8 Trainium2 NeuronCores.

Strategy: expert-parallel. The host computes the (tiny) gating network in
fp64 -- logits = x @ wg + bg, top-2, softmax -- and dispatches each token to
the cores owning its two selected experts (the "all-to-all dispatch tokens
by routing decision" sharding). Each core runs its expert's FFN
    y = relu(x_e @ w1[e] + b1[e]) @ w2[e]
over its gathered tokens (padded to a common static NT), scales rows by the
gate weight on-device, and the host scatter-adds the two slots per token
back together (plus the combine@b2 bias term).

All device inputs are host-permuted so that every SBUF partition's data is
one contiguous DRAM run (~8KB DMA lines -> 4KB packets -> high per-engine
DMA rate); without this the HWDGE engines move ~13 GB/s each on 1KB lines.

Hardcoded problem shape: x [4,4096,512], w1 [8,512,1024], w2 [8,1024,512],
wg [512,8], top_k=2.
"""

import os
import numpy as np

B, S, D, F, E = 4, 4096, 512, 1024, 8
TOP_K = 2
N_CORES = 8
KD = D // 128   # contraction blocks for mm1
FB = F // 128   # F blocks (h partition blocks / mm2 contraction blocks)

# matmul operand dtype mode: "bf16" | "f32" | "f32r"
DT_MODE = os.environ.get("MOE_DT", "bf16")
TRACE = os.environ.get("MOE_TRACE", "0") == "1"


def _chunk_plan(NT, mode):
    """Token chunk sizes (moving-dim tiles) and DMA groups.

    f32r matmuls with moving dim < 256 drop to 4 cycles/row, so keep
    chunks >= 256 there; bf16 has no such penalty and a small LAST chunk
    trims the post-last-matmul tail. Group 0 is a single chunk so the
    first x load is small (PE starts sooner); later groups pair chunks
    for fat DMA transfers.
    """
    rem = NT % 512
    chunks = [512] * (NT // 512)
    if mode == "bf16":
        if rem:
            chunks.append(rem)
    else:
        if rem == 128:
            chunks = chunks[:-1] + [384, 256]
        elif rem:
            chunks.append(rem)
        assert all(c >= 256 for c in chunks), chunks
    assert sum(chunks) == NT, chunks
    groups = [[0]]  # list of lists of chunk indices
    for i in range(1, len(chunks), 2):
        groups.append(list(range(i, min(i + 2, len(chunks)))))
    return chunks, groups


def _build_program(NT, mode):
    from concourse import bacc, tile, mybir

    dt = mybir.dt
    DT = {"bf16": dt.bfloat16, "f32": dt.float32, "f32r": dt.float32r}[mode]

    nc = bacc.Bacc("TRN2", target_bir_lowering=False, debug=False)

    chunks, groups = _chunk_plan(NT, mode)
    offs = [sum(chunks[:i]) for i in range(len(chunks) + 1)]

    # host-permuted inputs: per-partition contiguous runs
    xp_d = nc.dram_tensor("xp", [128, KD * NT], DT, kind="ExternalInput").ap()
    w1_d = nc.dram_tensor("w1p", [128, KD * F], DT, kind="ExternalInput").ap()
    w2_d = nc.dram_tensor("w2p", [128, FB * D], DT, kind="ExternalInput").ap()
    b1_d = nc.dram_tensor("b1c", [128, FB], dt.float32, kind="ExternalInput").ap()
    g_d = nc.dram_tensor("gate2", [128, NT // 128], dt.float32, kind="ExternalInput").ap()
    y_d = nc.dram_tensor("y", [NT, D], dt.float32, kind="ExternalOutput").ap()

    def psplit_dma(out_ap, in_ap, n, eng=None):
        # split a [128, W] transfer into n partition slices so several DMA
        # engines work it concurrently; eng picks the issuing engine's
        # HWDGE queue (sync and scalar are separate queue domains on TRN2)
        eng = eng or nc.sync
        step = 128 // n
        for j in range(n):
            eng.dma_start(
                out=out_ap[j * step:(j + 1) * step, :],
                in_=in_ap[j * step:(j + 1) * step, :],
            )

    with tile.TileContext(nc) as tc:
        with (
            tc.tile_pool(name="w", bufs=1) as wpool,
            tc.tile_pool(name="x", bufs=2) as xpool,
            tc.tile_pool(name="h", bufs=3) as hpool,
            tc.tile_pool(name="o", bufs=4) as opool,
            tc.tile_pool(name="ps1", bufs=4, space="PSUM") as ps1,
            tc.tile_pool(name="ps2", bufs=4, space="PSUM") as ps2,
        ):
            # DMA issue order = FIFO service order: w1 + x group 0 (needed
            # by the first matmuls), then w2, then later x groups. 4-way
            # [32-partition] splits are measured optimal: ~14 GB/s per
            # engine; narrower slices drop per-engine rate (SBUF ports) and
            # wider ones leave engines idle.
            w1_sb = wpool.tile([128, KD * F], DT)
            psplit_dma(w1_sb, w1_d, 4)

            x_tiles = {}

            def load_group(g):
                cidx = groups[g]
                goff = offs[cidx[0]]
                gs = sum(chunks[c] for c in cidx)
                x_sb = xpool.tile([128, KD * gs], DT, tag="x")
                psplit_dma(x_sb, xp_d[:, KD * goff:KD * (goff + gs)], 4)
                x_tiles[g] = x_sb

            load_group(0)

            w2_sb = wpool.tile([128, FB * D], DT)
            psplit_dma(w2_sb, w2_d, 4)
            b1_sb = wpool.tile([128, FB], dt.float32)
            nc.sync.dma_start(out=b1_sb[:], in_=b1_d[:])
            g_sb = wpool.tile([128, NT // 128], dt.float32)
            nc.sync.dma_start(out=g_sb[:], in_=g_d[:])

            for g, cidx in enumerate(groups):
                if g + 1 < len(groups):
                    load_group(g + 1)
                x_sb = x_tiles.pop(g)
                goff = offs[cidx[0]]
                gs = sum(chunks[c] for c in cidx)
                for c in cidx:
                    cs = chunks[c]
                    off = offs[c]
                    lo = off - goff  # chunk's token offset inside the group
                    h_sb = hpool.tile([128, FB, cs], DT, tag="h")
                    for fb in range(FB):
                        p = ps1.tile([128, cs], dt.float32, tag="ps1")
                        for kc in range(KD):
                            nc.tensor.matmul(
                                p[:],
                                w1_sb[:, kc * F + fb * 128:kc * F + (fb + 1) * 128],
                                x_sb[:, kc * gs + lo:kc * gs + lo + cs],
                                start=(kc == 0),
                                stop=(kc == KD - 1),
                            )
                        nc.scalar.activation(
                            h_sb[:, fb, :],
                            p[:],
                            mybir.ActivationFunctionType.Relu,
                            bias=b1_sb[:, fb:fb + 1],
                            scale=1.0,
                        )
                    for tb in range(cs // 128):
                        p2 = ps2.tile([128, 512], dt.float32, tag="ps2")
                        for fb in range(FB):
                            nc.tensor.matmul(
                                p2[:],
                                h_sb[:, fb, tb * 128:(tb + 1) * 128],
                                w2_sb[:, fb * D:(fb + 1) * D],
                                start=(fb == 0),
                                stop=(fb == FB - 1),
                            )
                        o_sb = opool.tile([128, 512], dt.float32, tag="o")
                        nc.vector.tensor_scalar_mul(
                            o_sb[:], p2[:], g_sb[:, off // 128 + tb:off // 128 + tb + 1]
                        )
                        nc.sync.dma_start(
                            out=y_d[off + tb * 128:off + (tb + 1) * 128, :],
                            in_=o_sb[:],
                        )
    nc.compile()
    return nc


def _install_ntff_hook():
    """Register the axon NTFF profiling hook that run_bass_kernel_spmd
    (trace=True) looks for under antenv.axon_hooks; this container's antenv
    lacks that module, so recreate it via ctypes against libaxon_pjrt.so."""
    import sys, types, ctypes, contextlib

    if "antenv.axon_hooks" in sys.modules:
        return
    try:
        lib = ctypes.CDLL("/opt/axon/libaxon_pjrt.so")
    except OSError:
        return
    if not hasattr(lib, "axon_start_nrt_profile"):
        return
    lib.axon_start_nrt_profile.argtypes = [ctypes.POINTER(ctypes.c_int64), ctypes.c_size_t]
    lib.axon_start_nrt_profile.restype = ctypes.c_int64
    lib.axon_stop_nrt_profile.argtypes = [ctypes.c_char_p]
    lib.axon_stop_nrt_profile.restype = ctypes.c_int64

    @contextlib.contextmanager
    def _hook(output_dir, device_ids):
        import jax

        jax.devices()
        if device_ids:
            ids = (ctypes.c_int64 * len(device_ids))(*device_ids)
            rc = lib.axon_start_nrt_profile(ids, len(device_ids))
        else:
            rc = lib.axon_start_nrt_profile(None, 0)
        if rc != 0:
            raise RuntimeError(f"axon_start_nrt_profile rc={rc}")
        try:
            yield
        finally:
            n = lib.axon_stop_nrt_profile(str(output_dir).encode())
            print(f"profile: {n} ntff file(s) written to {output_dir}")

    mod = types.ModuleType("antenv.axon_hooks")
    _holder = {"h": _hook}
    mod.set_axon_ntff_profile_hook = lambda h: _holder.__setitem__("h", h)
    mod.get_axon_ntff_profile_hook = lambda: _holder["h"]
    sys.modules["antenv.axon_hooks"] = mod

    # avoid the S3/Fish artifact upload in the trace post-processing path
    import concourse.bass_utils as bu

    bu.upload_artifacts = lambda tmpdir: str(tmpdir)


def kernel(**inputs):
    from concourse.bass_utils import run_bass_kernel_spmd

    if TRACE:
        _install_ntff_hook()

    x = np.asarray(inputs["x"], np.float32)
    w1 = np.asarray(inputs["w1"], np.float32)
    b1 = np.asarray(inputs["b1"], np.float32)
    w2 = np.asarray(inputs["w2"], np.float32)
    b2 = np.asarray(inputs["b2"], np.float32)
    wg = np.asarray(inputs["wg"], np.float32)
    bg = np.asarray(inputs["bg"], np.float32)

    T = x.shape[0] * x.shape[1]
    xf = x.reshape(T, D)

    # ---- host gating (fp64): logits -> top-2 (jax.lax.top_k tie order:
    # lower index wins -> stable argsort on -logits) -> softmax over top-2.
    logits = xf.astype(np.float64) @ wg.astype(np.float64) + bg.astype(np.float64)
    order = np.argsort(-logits, axis=1, kind="stable")
    top_idx = order[:, :TOP_K]                      # [T, K]
    top_vals = np.take_along_axis(logits, top_idx, axis=1)
    gwts = np.exp(top_vals - top_vals.max(axis=1, keepdims=True))
    gwts = gwts / gwts.sum(axis=1, keepdims=True)   # [T, K]

    # ---- dispatch: sort slots (t, k) by expert; per-expert contiguous runs.
    flat_expert = top_idx.ravel()                   # slot s = t*K + k
    perm = np.argsort(flat_expert, kind="stable")   # slots grouped by expert
    counts = np.bincount(flat_expert, minlength=E)
    cum = np.concatenate([[0], np.cumsum(counts)])
    slot_tok = perm // TOP_K                        # token of each sorted slot
    gates_sorted = gwts.ravel()[perm].astype(np.float32)

    NT = max(512, int(-(-counts.max() // 128)) * 128)
    NTG = NT // 128
    chunks, groups = _chunk_plan(NT, DT_MODE)
    offs = [sum(chunks[:i]) for i in range(len(chunks) + 1)]

    if DT_MODE == "bf16":
        import ml_dtypes
        io_dtype = ml_dtypes.bfloat16
    else:
        io_dtype = np.float32

    w1_io = w1.astype(io_dtype)
    w2_io = w2.astype(io_dtype)

    group_bounds = [(offs[ci[0]], offs[ci[-1] + 1]) for ci in groups]

    def permute_x(xt):
        # xt [D, NT] -> [128, sum_g KD*gs]: per partition, per group,
        # (kc, token) contiguous
        xr = xt.reshape(KD, 128, NT)
        parts = [
            xr[:, :, g0:g1].transpose(1, 0, 2).reshape(128, -1)
            for (g0, g1) in group_bounds
        ]
        return np.ascontiguousarray(np.concatenate(parts, axis=1))

    in_maps = []
    for e in range(E):
        n = int(counts[e])
        toks = slot_tok[cum[e]:cum[e] + n]
        xt = np.zeros((D, NT), io_dtype)
        xt[:, :n] = xf[toks].astype(io_dtype).T
        gate = np.zeros(NT, np.float32)
        gate[:n] = gates_sorted[cum[e]:cum[e] + n]
        in_maps.append({
            "xp": permute_x(xt),
            "w1p": np.ascontiguousarray(
                w1_io[e].reshape(KD, 128, F).transpose(1, 0, 2).reshape(128, KD * F)),
            "w2p": np.ascontiguousarray(
                w2_io[e].reshape(FB, 128, D).transpose(1, 0, 2).reshape(128, FB * D)),
            "b1c": np.ascontiguousarray(b1[e].reshape(FB, 128).T),
            "gate2": np.ascontiguousarray(gate.reshape(NTG, 128).T),
        })

    nc = _build_program(NT, DT_MODE)
    res = run_bass_kernel_spmd(nc, in_maps, list(range(N_CORES)), trace=TRACE)
    if TRACE and res.exec_time_ns is not None:
        print(f"HW exec time: {res.exec_time_ns} ns")

    # ---- unshard: scatter slots back, sum the K slots per token, add b2 term.
    out_slots = np.zeros((T * TOP_K, D), np.float32)
    for e in range(E):
        n = int(counts[e])
        out_slots[perm[cum[e]:cum[e] + n]] = res.results[e]["y"][:n]
    out = out_slots.reshape(T, TOP_K, D).sum(axis=1)

    # combine @ b2 (gate-weighted expert output biases)
    combine = np.zeros((T, E), np.float32)
    np.put_along_axis(combine, top_idx, gwts.astype(np.float32), axis=1)
    out += combine @ b2

    return out.reshape(B, S, D).astype(np.float32)
